# revision 20
# baseline (speedup 1.0000x reference)
"""DeepseekV2 MoE Trainium2 kernel (8 NeuronCores, expert-parallel).

Strategy
--------
Routing (a [T,16] softmax/top-k control plane, ~0.02% of FLOPs) is computed
on host, exactly replicating the reference semantics. The heavy data plane
runs on 8 cores:

  - Expert parallel: 16 routed experts -> 2 per core ("slot0"/"slot1").
    Host gathers each expert's routed tokens (transposed, bf16) so the
    device does dense grouped GEMMs with static shapes. Slot capacities
    C0/C1 are compile-time constants derived from the actual routing.
  - Shared expert: tensor-parallel over its intermediate dim (2816/8 = 352,
    zero-padded to 384 per core). Each core emits a partial [T,H] output.
  - Combine: per-token top-k weights are applied on-device (per-partition
    scalar multiply on the expert outputs); host scatter-adds routed rows
    and sums the 8 shared partials (the unshard for partial-sum sharding).

All matmuls run bf16 x bf16 -> fp32 PSUM. Weights are host-pretiled into
kernel-native layouts so every DMA is a large fully-contiguous transfer.
"""

import numpy as np
import ml_dtypes

import concourse.bacc as bacc
import concourse.bass as bass
import concourse.mybir as mybir
import concourse.tile as tile
from concourse.bass_utils import run_bass_kernel_spmd

BF16 = ml_dtypes.bfloat16
F32 = mybir.dt.float32
BF = mybir.dt.bfloat16

# model dims (hardcoded per problem spec)
H = 2048
I = 1408
E = 16
TOP_K = 4
N_GROUP = 4
TOPK_GROUP = 2
SCALE = 16.0
SI = 2816          # shared intermediate (2 * I)
B, S = 1, 2048
T = B * S
N_CORES = 8

KT = H // 128      # 16 k-tiles over hidden dim
IT = I // 128      # 11 i-tiles over routed intermediate
MT = 2 * I // 128  # 22 m-tiles over merged gate|up
HQ = 4             # H / 512 output column blocks
SIP = 384          # per-core shared intermediate, padded from 352
SIT = SIP // 128   # 3
SMT = 2 * SIP // 128  # 6 m-tiles over shared gate|up slice
TCH = T // 512     # 4 token chunks for shared stage 1

_PROGRAM_CACHE = {}
last_run_info = {}


# --------------------------------------------------------------------------
# host routing (exact replication of reference.py semantics)
# --------------------------------------------------------------------------

def _topk_desc_stable(a, k):
    idx = np.argsort(-a, axis=-1, kind="stable")[..., :k]
    return np.take_along_axis(a, idx, axis=-1), idx


def _compute_routing(hidden_states, gate_w):
    x = hidden_states.reshape(-1, H).astype(np.float32)
    logits = x @ gate_w.T.astype(np.float32)                  # [T, E]
    grouped = logits.reshape(T, N_GROUP, E // N_GROUP)
    group_scores = grouped.max(axis=-1)
    _, group_idx = _topk_desc_stable(group_scores, TOPK_GROUP)
    keep = np.zeros((T, N_GROUP), bool)
    rows = np.arange(T)[:, None]
    keep[rows, group_idx] = True
    grouped = np.where(keep[..., None], grouped, np.float32(0.0))
    logits = grouped.reshape(T, E)
    m = logits.max(axis=-1, keepdims=True)
    ex = np.exp(logits - m)
    probs = (ex / ex.sum(axis=-1, keepdims=True)).astype(np.float32)
    topk_w, topk_ids = _topk_desc_stable(probs, TOP_K)
    topk_w = topk_w * np.float32(SCALE)
    combine = np.zeros((T, E), np.float32)
    np.add.at(combine, (rows, topk_ids), topk_w)
    return combine


# --------------------------------------------------------------------------
# device program
# --------------------------------------------------------------------------

def _chunks(total, step):
    return [(o, min(step, total - o)) for o in range(0, total, step)]


def _balanced_chunks(total, step=512, align=32):
    """Split `total` into near-equal chunks of <= step, multiples of `align`
    (except possibly the last). Avoids tiny LDW-bound remainder chunks."""
    n = -(-total // step)
    base = -(-total // (n * align)) * align
    out = []
    o = 0
    while o < total:
        w = min(base, total - o)
        out.append((o, w))
        o += w
    return out


def _build_program(C0, C1):
    """One SPMD program for all 8 cores; C0/C1 = routed slot capacities."""
    nct0 = -(-C0 // 128)
    nct1 = -(-C1 // 128)
    S0P = nct0 * 128
    JTOT = S0P + nct1 * 128
    NTT = nct0 + nct1

    nc = bacc.Bacc("TRN2", target_bir_lowering=False, debug=False,
                   num_devices=N_CORES)

    def din(name, shape, dt=BF):
        return nc.dram_tensor(name, list(shape), dt, kind="ExternalInput").ap()

    def dout(name, shape, dt=BF):
        return nc.dram_tensor(name, list(shape), dt, kind="ExternalOutput").ap()

    xg0_d = din("xg0", [128, KT, S0P])
    xg1_d = din("xg1", [128, KT, JTOT - S0P])
    xt_d = din("xt", [TCH, 128, KT, 512])
    wgu0_d = din("wgu0", [MT, 128, KT, 128])
    wgu1_d = din("wgu1", [MT, 128, KT, 128])
    wd0_d = din("wd0", [HQ, 128, IT, 512])
    wd1_d = din("wd1", [HQ, 128, IT, 512])
    sgu_d = din("sgu", [SMT, 128, KT, 128])
    sdw_d = din("sdw", [HQ, 128, SIT, 512])
    cv_d = din("cv", [128, NTT], F32)
    yr_d = dout("yr", [HQ, JTOT, 512])
    ysh_d = dout("ysh", [HQ, T, 512])

    with tile.TileContext(nc) as tc:
        with tc.tile_pool(name="persist", bufs=1) as pp, \
             tc.tile_pool(name="wgu_pool", bufs=3) as wgup, \
             tc.tile_pool(name="wd_pool", bufs=3) as wdp, \
             tc.tile_pool(name="sdw_pool", bufs=2) as sdp, \
             tc.tile_pool(name="xw_pool", bufs=2) as xwp, \
             tc.tile_pool(name="out_pool", bufs=6) as op, \
             tc.tile_pool(name="ps1", bufs=2, space="PSUM") as ps1, \
             tc.tile_pool(name="ps2", bufs=6, space="PSUM") as ps2:

            xg0_sb = pp.tile([128, KT, S0P], BF, name="xg0_sb", tag="xg0_sb")
            xg1_sb = pp.tile([128, KT, JTOT - S0P], BF, name="xg1_sb",
                             tag="xg1_sb")
            cv_sb = pp.tile([128, NTT], F32, name="cv_sb", tag="cv_sb")
            sgu_sb = pp.tile([128, SMT, KT, 128], BF, name="sgu_sb", tag="sgu_sb")
            aT0 = pp.tile([128, IT, C0], BF, name="aT0", tag="aT0")
            aT1 = pp.tile([128, IT, C1], BF, name="aT1", tag="aT1")
            aTs = pp.tile([128, SIT, T], BF, name="aTs", tag="aTs")

            # ---- stage 1, routed slots: y^T = wgu^T-tiles @ xg, silu*mul ----
            def routed_stage1(wgu_d, aT, xg_sb, C):
                for m in range(MT):
                    wt = wgup.tile([128, KT, 128], BF, name="wt", tag="wgu")
                    nc.sync.dma_start(wt[:], wgu_d[m])
                    if m == 0 and wgu_d is wgu0_d:
                        # token DMAs queue on the sync ring right behind the
                        # first weight block; the queue then self-paces the
                        # remaining weight prefetches against m=0..2 compute
                        nc.sync.dma_start(xg0_sb[:], xg0_d[:])
                        nc.sync.dma_start(xg1_sb[:], xg1_d[:])
                        nc.sync.dma_start(cv_sb[:], cv_d[:])
                    for (c0, cw) in _balanced_chunks(C):
                        ps = ps1.tile([128, cw], F32, name="ps_s1", tag="ps1")
                        for k in range(KT):
                            nc.tensor.matmul(
                                ps[:], wt[:, k], xg_sb[:, k, c0:c0 + cw],
                                start=(k == 0), stop=(k == KT - 1))
                        if m < IT:
                            nc.scalar.activation(
                                aT[:, m, c0:c0 + cw], ps[:],
                                mybir.ActivationFunctionType.Silu)
                        else:
                            nc.vector.tensor_mul(
                                aT[:, m - IT, c0:c0 + cw],
                                aT[:, m - IT, c0:c0 + cw], ps[:])

            routed_stage1(wgu0_d, aT0, xg0_sb, C0)
            for m in range(SMT):
                nc.scalar.dma_start(sgu_sb[:, m], sgu_d[m])
            routed_stage1(wgu1_d, aT1, xg1_sb, C1)

            # ---- stage 1, shared expert (stream x^T chunks) ----
            # xw loads ride the scalar ring: the scheduler hoists them to
            # kernel start (slots are free), and on the sync ring they jam
            # the wgu weight stream the PE is actively consuming
            for tch in range(TCH):
                xw = xwp.tile([128, KT, 512], BF, name="xw", tag="xw")
                nc.scalar.dma_start(xw[:], xt_d[tch])
                for m in range(SMT):
                    ps = ps1.tile([128, 512], F32, name="ps_sh1", tag="ps1")
                    for k in range(KT):
                        nc.tensor.matmul(ps[:], sgu_sb[:, m, k], xw[:, k],
                                         start=(k == 0), stop=(k == KT - 1))
                    t0 = tch * 512
                    if m < SIT:
                        nc.scalar.activation(
                            aTs[:, m, t0:t0 + 512], ps[:],
                            mybir.ActivationFunctionType.Silu)
                    else:
                        nc.vector.tensor_mul(
                            aTs[:, m - SIT, t0:t0 + 512],
                            aTs[:, m - SIT, t0:t0 + 512], ps[:])

            # ---- stage 2, routed: out rows = aT^T @ wd, scale by combine ----
            def routed_stage2(wd_d, aT, rowoff, cvoff, C):
                for hq in range(HQ):
                    wdt = wdp.tile([128, IT, 512], BF, name="wdt", tag="wd")
                    nc.sync.dma_start(wdt[:], wd_d[hq])
                    for (r0, cp) in _chunks(C, 128):
                        ct = r0 // 128
                        ps = ps2.tile([128, 512], F32, name="ps_s2", tag="ps2")
                        for it in range(IT):
                            nc.tensor.matmul(
                                ps[:cp], aT[:, it, r0:r0 + cp], wdt[:, it],
                                start=(it == 0), stop=(it == IT - 1))
                        ot = op.tile([128, 512], BF, name="ot", tag="ot")
                        nc.vector.tensor_scalar_mul(
                            ot[:cp], ps[:cp],
                            cv_sb[:cp, cvoff + ct:cvoff + ct + 1])
                        nc.sync.dma_start(
                            yr_d[hq, rowoff + r0:rowoff + r0 + cp], ot[:cp])

            routed_stage2(wd0_d, aT0, 0, 0, C0)
            routed_stage2(wd1_d, aT1, S0P, nct0, C1)

            # ---- stage 2, shared partial ----
            for hq in range(HQ):
                sdt = sdp.tile([128, SIT, 512], BF, name="sdt", tag="sdw")
                nc.sync.dma_start(sdt[:], sdw_d[hq])
                for ct in range(T // 128):
                    r0 = ct * 128
                    ps = ps2.tile([128, 512], F32, name="ps_shs2", tag="ps2")
                    for it in range(SIT):
                        nc.tensor.matmul(ps[:], aTs[:, it, r0:r0 + 128],
                                         sdt[:, it],
                                         start=(it == 0), stop=(it == SIT - 1))
                    ot = op.tile([128, 512], BF, name="ot_sh", tag="ot")
                    # alternate copies across DVE/ACT: a single engine barely
                    # keeps up with the 3-matmul accumulation groups here
                    if ct % 2 == 0:
                        nc.vector.tensor_copy(ot[:], ps[:])
                    else:
                        nc.scalar.activation(ot[:], ps[:],
                                             mybir.ActivationFunctionType.Copy)
                    nc.sync.dma_start(ysh_d[hq, r0:r0 + 128], ot[:])

    nc.finalize()
    return nc


# --------------------------------------------------------------------------
# host data prep
# --------------------------------------------------------------------------

def _tile_wgu(w):  # [H, 2I] -> [MT, 128, KT, 128]
    return np.ascontiguousarray(
        w.reshape(KT, 128, MT, 128).transpose(2, 1, 0, 3))


def _tile_wd(w):   # [I, H] -> [HQ, 128, IT, 512]
    return np.ascontiguousarray(
        w.reshape(IT, 128, HQ, 512).transpose(2, 1, 0, 3))


def _tile_sgu(w):  # [H, 2*SIP] -> [SMT, 128, KT, 128]
    return np.ascontiguousarray(
        w.reshape(KT, 128, SMT, 128).transpose(2, 1, 0, 3))


def _tile_sdw(w):  # [SIP, H] -> [HQ, 128, SIT, 512]
    return np.ascontiguousarray(
        w.reshape(SIT, 128, HQ, 512).transpose(2, 1, 0, 3))


def kernel(hidden_states, gate_w, w_gate_up, w_down, shared_gate_up,
           shared_down, _trace=False):
    x = np.asarray(hidden_states, np.float32).reshape(T, H)
    combine = _compute_routing(np.asarray(hidden_states, np.float32),
                               np.asarray(gate_w, np.float32))

    idx_lists = [np.nonzero(combine[:, e] != 0.0)[0].astype(np.int64)
                 for e in range(E)]
    counts = np.array([len(ix) for ix in idx_lists])
    order = np.argsort(-counts, kind="stable")
    slot0_experts = [int(order[i]) for i in range(N_CORES)]
    slot1_experts = [int(order[2 * N_CORES - 1 - i]) for i in range(N_CORES)]

    C0 = max(32, int(-(-max(counts[e] for e in slot0_experts) // 32) * 32))
    C1 = max(32, int(-(-max(counts[e] for e in slot1_experts) // 32) * 32))
    nct0 = -(-C0 // 128)
    nct1 = -(-C1 // 128)
    S0P = nct0 * 128
    JTOT = S0P + nct1 * 128
    NTT = nct0 + nct1

    key = (C0, C1)
    if key not in _PROGRAM_CACHE:
        _PROGRAM_CACHE[key] = _build_program(C0, C1)
    nc = _PROGRAM_CACHE[key]

    xT16 = np.ascontiguousarray(x.T).astype(BF16)              # [H, T]
    # [TCH, 128, KT, 512] view of x^T
    xt_tiled = np.ascontiguousarray(
        xT16.reshape(KT, 128, TCH, 512).transpose(2, 1, 0, 3))

    wgu16 = np.asarray(w_gate_up, np.float32).astype(BF16)
    wd16 = np.asarray(w_down, np.float32).astype(BF16)
    sgu16 = np.asarray(shared_gate_up, np.float32).astype(BF16)
    sdw16 = np.asarray(shared_down, np.float32).astype(BF16)

    in_maps = []
    meta = []
    for c in range(N_CORES):
        e0, e1 = slot0_experts[c], slot1_experts[c]
        xg0 = np.zeros((128, KT, S0P), BF16)
        xg1 = np.zeros((128, KT, JTOT - S0P), BF16)
        cvt = np.zeros((NTT * 128,), np.float32)
        for s, (e, xg) in enumerate([(e0, xg0), (e1, xg1)]):
            ix = idx_lists[e]
            g = xT16[:, ix].reshape(KT, 128, len(ix)).transpose(1, 0, 2)
            xg[:, :, :len(ix)] = g
            cvoff = 0 if s == 0 else nct0 * 128
            cvt[cvoff:cvoff + len(ix)] = combine[ix, e]
        cv_t = np.ascontiguousarray(cvt.reshape(NTT, 128).T)

        # shared expert TP slice (352 wide, zero-padded to SIP=384)
        lo = c * 352
        sgu_sl = np.zeros((H, 2 * SIP), BF16)
        sgu_sl[:, :352] = sgu16[:, lo:lo + 352]
        sgu_sl[:, SIP:SIP + 352] = sgu16[:, SI + lo:SI + lo + 352]
        sdw_sl = np.zeros((SIP, H), BF16)
        sdw_sl[:352] = sdw16[lo:lo + 352]

        in_maps.append({
            "xg0": xg0,
            "xg1": xg1,
            "xt": xt_tiled,
            "wgu0": _tile_wgu(wgu16[e0]),
            "wgu1": _tile_wgu(wgu16[e1]),
            "wd0": _tile_wd(wd16[e0]),
            "wd1": _tile_wd(wd16[e1]),
            "sgu": _tile_sgu(sgu_sl),
            "sdw": _tile_sdw(sdw_sl),
            "cv": cv_t,
        })
        meta.append((e0, e1))

    res = run_bass_kernel_spmd(nc, in_maps, list(range(N_CORES)),
                               trace=_trace)
    last_run_info["exec_time_ns"] = res.exec_time_ns
    last_run_info["profile_json"] = res.profile_json
    last_run_info["results"] = res

    # ---- host combine (unshard) ----
    out = np.zeros((T, H), np.float32)
    all_idx = []
    all_rows = []
    for c in range(N_CORES):
        yr = np.asarray(res.results[c]["yr"], dtype=BF16)   # [HQ, JTOT, 512]
        ysh = np.asarray(res.results[c]["ysh"], dtype=BF16)
        out += ysh.transpose(1, 0, 2).reshape(T, H).astype(np.float32)
        yr_full = yr.transpose(1, 0, 2).reshape(JTOT, H).astype(np.float32)
        e0, e1 = meta[c]
        for (e, off) in [(e0, 0), (e1, S0P)]:
            ix = idx_lists[e]
            all_idx.append(ix)
            all_rows.append(yr_full[off:off + len(ix)])
    all_idx = np.concatenate(all_idx)
    all_rows = np.concatenate(all_rows, axis=0)
    if len(all_idx) == TOP_K * T:
        perm = np.argsort(all_idx, kind="stable")
        out += all_rows[perm].reshape(T, TOP_K, H).sum(axis=1)
    else:  # fallback for degenerate routing (a token with <4 experts)
        np.add.at(out, all_idx, all_rows)

    return out.reshape(B, S, H).astype(np.float32)
